# revision 24
# baseline (speedup 1.0000x reference)
"""GPT-OSS MoE experts kernel for Trainium2 (8 NeuronCores, expert-parallel).

Strategy
--------
- Expert-parallel: core e owns expert e's weights (1/8 of total weight bytes,
  read exactly once -> memory-bound). Host does routing (gather tokens per
  expert), weight re-staging (slice expert, transpose to contraction-major
  [K, N] tile layout, cast fp16), and the final scatter-add combine. No
  collectives needed.
- The reference's per-32-block fp8 quant-dequant collapses exactly to
  "round each element to 4 significant bits (RTNE)": the block scale is a
  power of two (mantissa rounding is scale-invariant) and the +-448 clip can
  never bind by construction. On device this is 3 VectorE ops (Veltkamp
  split); the 4-significant-bit activation values are then EXACT in fp16.
- fp16 weights round at 2^-11; end-to-end error vs the f32 reference is
  ~7e-3 absmax-rel - dominated by quantization-boundary flips from layer-1
  perturbations - and fp16 halves the weight traffic of this DMA-bound
  kernel.
- Form-B matmuls: weight [K-part, n] tiles are the STATIONARY operand, ALL
  tokens ride the moving free dim (ccap <= 512). Outputs land output-major
  ([n, tokens]), which feeds layer 2 directly - no on-chip transposes.
- No padding waste: the contraction splits into 22 full 128-tiles plus a
  single [65, 2880] "k-tail strip" per matrix (rows 2816..2879 + the bias
  row, with the activations carrying a ones row at tail partition 64); the
  output dim splits into 22 full 128-tiles plus one 64-wide tail tile.
  Weight bytes per core = 3 * 2881*2880*2B = 49.8 MB (vs 52.0 padded).
- gate and up slabs are staged interleaved so each layer-1 n-tile is ONE
  contiguous ~1.4 MB DMA (11264 B per partition line).
- y accumulates in one resident SBUF tile, stored in three big chunks on the
  Scalar HWDGE ring: store completions must never gate the (Sync-ring)
  weight-load stream, because the 8 shared DMA-completion lanes round-robin
  over ALL HWDGE DMAs and a lingering store blocks later loads on its lane.
- Measured: the weight stream sustains ~420 GB/s through layer 1 and the
  whole-kernel DMA averages ~358 GB/s (the per-NC HBM share); ~6 us fixed
  program preamble and ~9 us drain/barrier postamble bracket the stream.
"""

import functools
import sys

sys.path.insert(0, "/opt/trn_rl_repo")

import numpy as np

import concourse.bass as bass  # noqa: F401
import concourse.mybir as mybir
import concourse.tile as tile
from concourse import bacc
from concourse.bass_utils import run_bass_kernel_spmd

P = 128
H = 2880          # hidden dim
II = 2880         # intermediate dim (gate/up width)
NE = 8            # experts == cores
KF = 22           # full 128-tiles over the contraction dim (22*128 = 2816)
KTAIL = 65        # k-tail strip: rows 2816..2879 + bias/ones row
NT = 23           # output tiles: 22 full + one 64-wide tail
NWT = 64          # width of the output tail tile
VC = float(2 ** 20 + 1)   # Veltkamp constant: RTNE to 4 significant bits
MAXTOK = 512              # moving free-dim (= PSUM f32 bank) limit

f32 = mybir.dt.float32
f16 = mybir.dt.float16
AF = mybir.ActivationFunctionType
ALU = mybir.AluOpType


def _rtne4(x):
    """Round f32 elements to 4 significant bits, RTNE (== reference
    quant_dequant_fp8 up to e4m3-subnormal leftovers)."""
    c = np.float32(VC)
    t = (x * c).astype(np.float32)
    return (t - (t - x)).astype(np.float32)


@functools.lru_cache(maxsize=4)
def _build(ccap):
    """Per-core Bass program; ccap = padded token capacity (<= MAXTOK)."""
    nc = bacc.Bacc(None, target_bir_lowering=False)

    xt_d = nc.declare_dram_parameter("xt", [P, KF, ccap], f16, isOutput=False)
    xtl_d = nc.declare_dram_parameter("xtl", [KTAIL, ccap], f16, isOutput=False)
    w1m_d = nc.declare_dram_parameter("w1m", [KF, P, 2, KF, P], f16, isOutput=False)
    w1n_d = nc.declare_dram_parameter("w1n", [P, 2, KF, NWT], f16, isOutput=False)
    w1k_d = nc.declare_dram_parameter("w1k", [KTAIL, 2, 2880], f16, isOutput=False)
    w2m_d = nc.declare_dram_parameter("w2m", [KF, P, KF, P], f16, isOutput=False)
    w2n_d = nc.declare_dram_parameter("w2n", [P, KF, NWT], f16, isOutput=False)
    w2k_d = nc.declare_dram_parameter("w2k", [KTAIL, 2880], f16, isOutput=False)
    wr_d = nc.declare_dram_parameter("wr", [P, ccap], f32, isOutput=False)
    y_d = nc.declare_dram_parameter("y", [P, NT, ccap], f16, isOutput=True)

    with tile.TileContext(nc) as tc:
        with (
            tc.tile_pool(name="consts", bufs=1) as consts,
            tc.tile_pool(name="w1slab", bufs=10) as w1pool,
            tc.tile_pool(name="w2slab", bufs=8) as w2pool,
            tc.tile_pool(name="tmp", bufs=2) as tmp,
            tc.tile_pool(name="psum", bufs=4, space="PSUM") as psum,
        ):
            # resident tensors; the input prologue rides the Scalar HWDGE
            # ring so the Sync ring starts streaming weight slabs at t=0
            xts = consts.tile([P, KF, ccap], f16, tag="xt", name="xt")
            nc.scalar.dma_start(xts, xt_d[:])
            xtl = consts.tile([KTAIL, ccap], f16, tag="xtl", name="xtl")
            nc.scalar.dma_start(xtl, xtl_d[:])
            wrep = consts.tile([P, ccap], f32, tag="wrep", name="wrep")
            nc.scalar.dma_start(wrep, wr_d[:])
            kt1 = consts.tile([KTAIL, 2, 2880], f16, tag="kt1", name="kt1")
            interT = consts.tile([P, KF, ccap], f16, tag="interT", name="interT")
            intertail = consts.tile([KTAIL, ccap], f16, tag="intertail",
                                    name="intertail")
            ysb = consts.tile([P, NT, ccap], f16, tag="ysb", name="ysb")

            # HAM warmup while the first slabs + xt stream in
            wtile = consts.tile([P, P], f16, tag="wtile", name="wtile")
            nc.vector.memset(wtile, 0.25)
            wup = psum.tile([P, ccap], f32, tag="ps_g", name="wup")
            for _ in range(48):
                nc.tensor.matmul(wup[:, :P], wtile, wtile,
                                 start=True, stop=True, skip_group_check=True)

            # ---- layer 1 + swiglu + rtne4, one n-tile at a time ----
            for nt in range(NT):
                nw = P if nt < KF else NWT
                if nt < KF:
                    slab = w1pool.tile([P, 2, KF, P], f16, tag="w1slab",
                                       name="w1slab")
                    nc.sync.dma_start(slab, w1m_d[nt])
                else:
                    slab = w1pool.tile([P, 2, KF, NWT], f16, tag="w1slab",
                                       name="w1slab")
                    nc.sync.dma_start(slab, w1n_d[:])
                if nt == 0:
                    # FIFO position: after slab0, before slab1 (needed by the
                    # 45th MM of iteration 0 -> arrives in time either way)
                    nc.sync.dma_start(kt1, w1k_d[:])
                gps = psum.tile([nw, ccap], f32, tag="ps_g", name="ps_g")
                ups = psum.tile([nw, ccap], f32, tag="ps_u", name="ps_u")
                for k in range(KF):
                    nc.tensor.matmul(gps, slab[:, 0, k, :], xts[:, k, :],
                                     start=(k == 0), stop=False)
                    nc.tensor.matmul(ups, slab[:, 1, k, :], xts[:, k, :],
                                     start=(k == 0), stop=False)
                nc.tensor.matmul(gps, kt1[:, 0, nt * P : nt * P + nw], xtl,
                                 start=False, stop=True)
                nc.tensor.matmul(ups, kt1[:, 1, nt * P : nt * P + nw], xtl,
                                 start=False, stop=True)
                # swiglu: gate=min(G,7); up1=clip(U,-7,7)+1; x=gate*sig(1.702g)*up1
                gate = tmp.tile([nw, ccap], f32, tag="t_gate", name="t_gate")
                nc.vector.tensor_scalar_min(gate, gps, 7.0)
                sig = tmp.tile([nw, ccap], f32, tag="t_sig", name="t_sig")
                nc.scalar.activation(sig, gate, AF.Sigmoid, scale=1.702)
                up1 = tmp.tile([nw, ccap], f32, tag="t_up", name="t_up")
                nc.vector.tensor_scalar(up1, ups, 1.0, -6.0, ALU.add, ALU.max)
                nc.vector.tensor_scalar_min(up1, up1, 8.0)
                nc.vector.tensor_mul(gate, gate, sig)          # gate*sig
                xv = tmp.tile([nw, ccap], f32, tag="t_xv", name="t_xv")
                nc.vector.tensor_mul(xv, gate, up1)            # x = swiglu
                tv = tmp.tile([nw, ccap], f32, tag="t_tv", name="t_tv")
                nc.vector.tensor_scalar_mul(tv, xv, VC)        # t = x*c
                nc.vector.tensor_sub(xv, tv, xv)               # d = t-x
                dst = interT[:, nt, :] if nt < KF else intertail[:NWT, :]
                nc.vector.tensor_sub(dst, tv, xv)              # rtne4 = t-d
            # layer-2 ones row rides tail partition 64 (pairs w2's bias row)
            nc.vector.memset(intertail[NWT : NWT + 1, :], 1.0)

            # kt2 load sits here in FIFO order: after all w1 slabs, before w2
            kt2 = consts.tile([KTAIL, 2880], f16, tag="kt2", name="kt2")
            nc.sync.dma_start(kt2, w2k_d[:])

            # ---- layer 2 + routing-weight scale ----
            YSPLIT = (11, 19, 22)   # store y in 3 chunks; last one is tiny
            for ht in range(NT):
                nw = P if ht < KF else NWT
                if ht < KF:
                    slab2 = w2pool.tile([P, KF, P], f16, tag="w2slab",
                                        name="w2slab")
                    nc.sync.dma_start(slab2, w2m_d[ht])
                else:
                    slab2 = w2pool.tile([P, KF, NWT], f16, tag="w2slab",
                                        name="w2slab")
                    nc.sync.dma_start(slab2, w2n_d[:])
                yps = psum.tile([nw, ccap], f32, tag="ps_g", name="ps_g")
                for k in range(KF):
                    nc.tensor.matmul(yps, slab2[:, k, :], interT[:, k, :],
                                     start=(k == 0), stop=False)
                nc.tensor.matmul(yps, kt2[:, ht * P : ht * P + nw], intertail,
                                 start=False, stop=True)
                nc.vector.tensor_mul(ysb[:nw, ht, :], yps, wrep[:nw, :])
                if ht in YSPLIT:
                    lo = 0 if ht <= YSPLIT[0] else YSPLIT[YSPLIT.index(ht) - 1] + 1
                    nc.scalar.dma_start(y_d[:, lo : ht + 1, :],
                                        ysb[:, lo : ht + 1, :])

    nc.finalize()
    return nc


def _stage(inputs):
    """Host-side routing + weight re-staging. Returns (nc, passes, assigns, T)."""
    hs = np.ascontiguousarray(np.asarray(inputs["hidden_states"], dtype=np.float32))
    ri = np.asarray(inputs["router_indices"]).astype(np.int64)
    rw = np.asarray(inputs["routing_weights"], dtype=np.float32)
    gup = np.asarray(inputs["gate_up_proj"], dtype=np.float32)
    gub = np.asarray(inputs["gate_up_proj_bias"], dtype=np.float32)
    dn = np.asarray(inputs["down_proj"], dtype=np.float32)
    dnb = np.asarray(inputs["down_proj_bias"], dtype=np.float32)

    T = hs.shape[0]
    topk = ri.shape[1]

    flat_e = ri.reshape(-1)
    order = np.argsort(flat_e, kind="stable")
    counts = np.bincount(flat_e, minlength=NE)
    starts = np.zeros(NE + 1, np.int64)
    starts[1:] = np.cumsum(counts)
    maxc = int(counts.max())
    # Each pass handles up to MAXTOK tokens per expert (seed-0 loads are ~142,
    # so this is a single pass; multiple passes only for pathological routing).
    npass = max(1, -(-maxc // MAXTOK))
    percap = -(-maxc // npass)
    ccap = max(32, -(-percap // 32) * 32)

    x_dq = _rtne4(hs).astype(np.float16)   # 4-sig-bit values: exact in fp16
    rw_flat = rw.reshape(-1)

    def stage_w(mat_t, bias):
        # mat_t: [K <= 2880, N <= 2880] contraction-major -> (main, ntail, ktail)
        w = np.zeros((2881, 2880), np.float16)
        w[: mat_t.shape[0], : mat_t.shape[1]] = mat_t.astype(np.float16)
        w[2880, : bias.shape[0]] = bias.astype(np.float16)
        main = w[: KF * P].reshape(KF, P, 2880)          # [kt, kp, n]
        wm = np.ascontiguousarray(
            main[:, :, : KF * P].reshape(KF, P, KF, P).transpose(2, 1, 0, 3)
        )                                                 # [nt, kp, kt, np]
        wn = np.ascontiguousarray(main[:, :, KF * P :].transpose(1, 0, 2))
        wk = np.ascontiguousarray(w[KF * P :])            # [65, 2880]
        return wm, wn, wk

    passes, assigns = [], []
    weights = []
    for e in range(NE):
        gm, gn, gk = stage_w(gup[e, 0::2, :].T, gub[e, 0::2])
        um, un, uk = stage_w(gup[e, 1::2, :].T, gub[e, 1::2])
        dm, dnn, dk = stage_w(dn[e].T, dnb[e])
        weights.append(dict(
            w1m=np.ascontiguousarray(np.stack([gm, um], axis=2)),
            w1n=np.ascontiguousarray(np.stack([gn, un], axis=1)),
            w1k=np.ascontiguousarray(np.stack([gk, uk], axis=1)),
            w2m=dm, w2n=dnn, w2k=dk,
        ))
    for p in range(npass):
        in_maps, passigns = [], []
        for e in range(NE):
            a_all = order[starts[e] : starts[e + 1]]
            a = a_all[p * ccap : (p + 1) * ccap]
            toks = a // topk
            ce = len(a)
            passigns.append((a, toks))

            xt_full = x_dq[toks].T                        # [2880, ce]
            xt = np.zeros((P, KF, ccap), np.float16)
            xt[:, :, :ce] = xt_full[: KF * P].reshape(KF, P, ce).transpose(1, 0, 2)
            xtl = np.zeros((KTAIL, ccap), np.float16)
            xtl[:NWT, :ce] = xt_full[KF * P :]
            xtl[NWT, :] = np.float16(1.0)

            wr_rep = np.zeros((P, ccap), np.float32)
            wr_rep[:, :ce] = rw_flat[a][None, :]

            in_maps.append(dict(xt=xt, xtl=xtl, wr=wr_rep, **weights[e]))
        passes.append(in_maps)
        assigns.append(passigns)

    nc = _build(ccap)
    return nc, passes, assigns, T


def kernel(**inputs):
    nc, passes, assigns, T = _stage(inputs)
    out = np.zeros((T, H), np.float32)
    for in_maps, passigns in zip(passes, assigns):
        res = run_bass_kernel_spmd(nc, in_maps, list(range(NE)))
        for e in range(NE):
            a, toks = passigns[e]
            if len(a):
                yt = res.results[e]["y"]            # [P, NT, ccap]
                yt = yt.transpose(1, 0, 2).reshape(NT * P, -1)[:H, : len(a)]
                np.add.at(out, toks, yt.T.astype(np.float32))
    return out


# revision 27
# speedup vs baseline: 1.0535x; 1.0535x over previous
"""GPT-OSS MoE experts kernel for Trainium2 (8 NeuronCores, expert-parallel).

Strategy
--------
- Expert-parallel: core e owns expert e's weights (1/8 of total weight bytes,
  read exactly once -> memory-bound). Host does routing (gather tokens per
  expert), weight re-staging (slice expert, transpose to contraction-major
  [K, N] tile layout, cast fp16), and the final scatter-add combine. No
  collectives needed.
- The reference's per-32-block fp8 quant-dequant collapses exactly to
  "round each element to 4 significant bits (RTNE)": the block scale is a
  power of two (mantissa rounding is scale-invariant) and the +-448 clip can
  never bind by construction. On device this is 3 VectorE ops (Veltkamp
  split); the 4-significant-bit activation values are then EXACT in fp16.
- fp16 weights round at 2^-11; end-to-end error vs the f32 reference is
  ~7e-3 absmax-rel - dominated by quantization-boundary flips from layer-1
  perturbations - and fp16 halves the weight traffic of this DMA-bound
  kernel.
- Form-B matmuls: weight [K-part, n] tiles are the STATIONARY operand, ALL
  tokens ride the moving free dim (ccap <= 512). Outputs land output-major
  ([n, tokens]), which feeds layer 2 directly - no on-chip transposes.
- No padding waste: the contraction splits into 22 full 128-tiles plus a
  single [65, 2880] "k-tail strip" per matrix (rows 2816..2879 + the bias
  row, with the activations carrying a ones row at tail partition 64); the
  output dim splits into 22 full 128-tiles plus one 64-wide tail tile.
  Weight bytes per core = 3 * 2881*2880*2B = 49.8 MB (vs 52.0 padded).
- gate and up slabs are staged interleaved so each layer-1 n-tile is ONE
  contiguous ~1.4 MB DMA (11264 B per partition line).
- y accumulates in one resident SBUF tile, stored in three big chunks on the
  Scalar HWDGE ring: store completions must never gate the (Sync-ring)
  weight-load stream, because the 8 shared DMA-completion lanes round-robin
  over ALL HWDGE DMAs and a lingering store blocks later loads on its lane.
- Measured: the weight stream sustains ~420 GB/s through layer 1 and the
  whole-kernel DMA averages ~358 GB/s (the per-NC HBM share); ~6 us fixed
  program preamble and ~9 us drain/barrier postamble bracket the stream.
"""

import functools
import sys

sys.path.insert(0, "/opt/trn_rl_repo")

import numpy as np

import concourse.bass as bass  # noqa: F401
import concourse.mybir as mybir
import concourse.tile as tile
from concourse import bacc
from concourse.bass_utils import run_bass_kernel_spmd

P = 128
H = 2880          # hidden dim
II = 2880         # intermediate dim (gate/up width)
NE = 8            # experts == cores
KF = 22           # full 128-tiles over the contraction dim (22*128 = 2816)
KTAIL = 65        # k-tail strip: rows 2816..2879 + bias/ones row
NT = 23           # output tiles: 22 full + one 64-wide tail
NWT = 64          # width of the output tail tile
VC = float(2 ** 20 + 1)   # Veltkamp constant: RTNE to 4 significant bits
MAXTOK = 512              # moving free-dim (= PSUM f32 bank) limit

f32 = mybir.dt.float32
f16 = mybir.dt.float16
AF = mybir.ActivationFunctionType
ALU = mybir.AluOpType


def _rtne4(x):
    """Round f32 elements to 4 significant bits, RTNE (== reference
    quant_dequant_fp8 up to e4m3-subnormal leftovers)."""
    c = np.float32(VC)
    t = (x * c).astype(np.float32)
    return (t - (t - x)).astype(np.float32)


@functools.lru_cache(maxsize=4)
def _build(ccap):
    """Per-core Bass program; ccap = padded token capacity (<= MAXTOK)."""
    nc = bacc.Bacc(None, target_bir_lowering=False)

    xt_d = nc.declare_dram_parameter("xt", [P, KF, ccap], f16, isOutput=False)
    xtl_d = nc.declare_dram_parameter("xtl", [KTAIL, ccap], f16, isOutput=False)
    w1m_d = nc.declare_dram_parameter("w1m", [KF, P, 2, KF, P], f16, isOutput=False)
    w1n_d = nc.declare_dram_parameter("w1n", [P, 2, KF, NWT], f16, isOutput=False)
    w1k_d = nc.declare_dram_parameter("w1k", [KTAIL, 2, 2880], f16, isOutput=False)
    w2m_d = nc.declare_dram_parameter("w2m", [KF, P, KF, P], f16, isOutput=False)
    w2n_d = nc.declare_dram_parameter("w2n", [P, KF, NWT], f16, isOutput=False)
    w2k_d = nc.declare_dram_parameter("w2k", [KTAIL, 2880], f16, isOutput=False)
    wr_d = nc.declare_dram_parameter("wr", [P, ccap], f32, isOutput=False)
    y_d = nc.declare_dram_parameter("y", [P, NT, ccap], f16, isOutput=True)

    with tile.TileContext(nc) as tc:
        with (
            tc.tile_pool(name="consts", bufs=1) as consts,
            tc.tile_pool(name="w1slab", bufs=10) as w1pool,
            tc.tile_pool(name="w2slab", bufs=8) as w2pool,
            tc.tile_pool(name="tmp", bufs=3) as tmp,
            tc.tile_pool(name="psum", bufs=4, space="PSUM") as psum,
        ):
            # resident tensors; the input prologue rides the Scalar HWDGE
            # ring so the Sync ring starts streaming weight slabs at t=0
            xts = consts.tile([P, KF, ccap], f16, tag="xt", name="xt")
            nc.scalar.dma_start(xts, xt_d[:])
            xtl = consts.tile([KTAIL, ccap], f16, tag="xtl", name="xtl")
            nc.scalar.dma_start(xtl, xtl_d[:])
            wrep = consts.tile([P, ccap], f32, tag="wrep", name="wrep")
            nc.scalar.dma_start(wrep, wr_d[:])
            kt1 = consts.tile([KTAIL, 2, 2880], f16, tag="kt1", name="kt1")
            interT = consts.tile([P, KF, ccap], f16, tag="interT", name="interT")
            intertail = consts.tile([KTAIL, ccap], f16, tag="intertail",
                                    name="intertail")
            ysb = consts.tile([P, NT, ccap], f16, tag="ysb", name="ysb")

            # HAM warmup while the first slabs + xt stream in
            wtile = consts.tile([P, P], f16, tag="wtile", name="wtile")
            nc.vector.memset(wtile, 0.25)
            wup = psum.tile([P, ccap], f32, tag="ps_g", name="wup")
            for _ in range(48):
                nc.tensor.matmul(wup[:, :P], wtile, wtile,
                                 start=True, stop=True, skip_group_check=True)

            # ---- layer 1 + swiglu + rtne4, one n-tile at a time ----
            # weight slabs alternate between the two HWDGE rings (Sync and
            # Scalar) so each ring's in-order descriptor stream is half as
            # dense - measured: single-ring streaming delivered every 3rd
            # slab ~1.5us late, stalling the (WAR-coupled) pipeline
            for nt in range(NT):
                nw = P if nt < KF else NWT
                eng = nc.sync if nt % 2 == 0 else nc.scalar
                if nt < KF:
                    slab = w1pool.tile([P, 2, KF, P], f16, tag="w1slab",
                                       name="w1slab")
                    eng.dma_start(slab, w1m_d[nt])
                else:
                    slab = w1pool.tile([P, 2, KF, NWT], f16, tag="w1slab",
                                       name="w1slab")
                    eng.dma_start(slab, w1n_d[:])
                if nt == 0:
                    # FIFO position: after slab0, before slab1 (needed by the
                    # 45th MM of iteration 0 -> arrives in time either way)
                    nc.sync.dma_start(kt1, w1k_d[:])
                gps = psum.tile([nw, ccap], f32, tag="ps_g", name="ps_g")
                ups = psum.tile([nw, ccap], f32, tag="ps_u", name="ps_u")
                for k in range(KF):
                    nc.tensor.matmul(gps, slab[:, 0, k, :], xts[:, k, :],
                                     start=(k == 0), stop=False)
                    nc.tensor.matmul(ups, slab[:, 1, k, :], xts[:, k, :],
                                     start=(k == 0), stop=False)
                nc.tensor.matmul(gps, kt1[:, 0, nt * P : nt * P + nw], xtl,
                                 start=False, stop=True)
                nc.tensor.matmul(ups, kt1[:, 1, nt * P : nt * P + nw], xtl,
                                 start=False, stop=True)
                # swiglu: gate=min(G,7); up1=clip(U,-7,7)+1; x=gate*sig(1.702g)*up1
                gate = tmp.tile([nw, ccap], f32, tag="t_gate", name="t_gate")
                nc.vector.tensor_scalar_min(gate, gps, 7.0)
                sig = tmp.tile([nw, ccap], f32, tag="t_sig", name="t_sig")
                nc.scalar.activation(sig, gate, AF.Sigmoid, scale=1.702)
                up1 = tmp.tile([nw, ccap], f32, tag="t_up", name="t_up")
                nc.vector.tensor_scalar(up1, ups, 1.0, -6.0, ALU.add, ALU.max)
                nc.vector.tensor_scalar_min(up1, up1, 8.0)
                nc.vector.tensor_mul(gate, gate, sig)          # gate*sig
                xv = tmp.tile([nw, ccap], f32, tag="t_xv", name="t_xv")
                nc.vector.tensor_mul(xv, gate, up1)            # x = swiglu
                tv = tmp.tile([nw, ccap], f32, tag="t_tv", name="t_tv")
                nc.vector.tensor_scalar_mul(tv, xv, VC)        # t = x*c
                nc.vector.tensor_sub(xv, tv, xv)               # d = t-x
                dst = interT[:, nt, :] if nt < KF else intertail[:NWT, :]
                nc.vector.tensor_sub(dst, tv, xv)              # rtne4 = t-d
            # layer-2 ones row rides tail partition 64 (pairs w2's bias row)
            nc.vector.memset(intertail[NWT : NWT + 1, :], 1.0)

            # kt2 load sits here in FIFO order: after all w1 slabs, before w2
            kt2 = consts.tile([KTAIL, 2880], f16, tag="kt2", name="kt2")
            nc.sync.dma_start(kt2, w2k_d[:])

            # ---- layer 2 + routing-weight scale ----
            YSPLIT = (11, 19, 22)   # store y in 3 chunks; last one is tiny
            for ht in range(NT):
                nw = P if ht < KF else NWT
                eng = nc.sync if ht % 2 == 0 else nc.scalar
                if ht < KF:
                    slab2 = w2pool.tile([P, KF, P], f16, tag="w2slab",
                                        name="w2slab")
                    eng.dma_start(slab2, w2m_d[ht])
                else:
                    slab2 = w2pool.tile([P, KF, NWT], f16, tag="w2slab",
                                        name="w2slab")
                    eng.dma_start(slab2, w2n_d[:])
                yps = psum.tile([nw, ccap], f32, tag="ps_g", name="ps_g")
                for k in range(KF):
                    nc.tensor.matmul(yps, slab2[:, k, :], interT[:, k, :],
                                     start=(k == 0), stop=False)
                nc.tensor.matmul(yps, kt2[:, ht * P : ht * P + nw], intertail,
                                 start=False, stop=True)
                nc.vector.tensor_mul(ysb[:nw, ht, :], yps, wrep[:nw, :])
                if ht in YSPLIT:
                    lo = 0 if ht <= YSPLIT[0] else YSPLIT[YSPLIT.index(ht) - 1] + 1
                    nc.scalar.dma_start(y_d[:, lo : ht + 1, :],
                                        ysb[:, lo : ht + 1, :])

    nc.finalize()
    return nc


def _stage(inputs):
    """Host-side routing + weight re-staging. Returns (nc, passes, assigns, T)."""
    hs = np.ascontiguousarray(np.asarray(inputs["hidden_states"], dtype=np.float32))
    ri = np.asarray(inputs["router_indices"]).astype(np.int64)
    rw = np.asarray(inputs["routing_weights"], dtype=np.float32)
    gup = np.asarray(inputs["gate_up_proj"], dtype=np.float32)
    gub = np.asarray(inputs["gate_up_proj_bias"], dtype=np.float32)
    dn = np.asarray(inputs["down_proj"], dtype=np.float32)
    dnb = np.asarray(inputs["down_proj_bias"], dtype=np.float32)

    T = hs.shape[0]
    topk = ri.shape[1]

    flat_e = ri.reshape(-1)
    order = np.argsort(flat_e, kind="stable")
    counts = np.bincount(flat_e, minlength=NE)
    starts = np.zeros(NE + 1, np.int64)
    starts[1:] = np.cumsum(counts)
    maxc = int(counts.max())
    # Each pass handles up to MAXTOK tokens per expert (seed-0 loads are ~142,
    # so this is a single pass; multiple passes only for pathological routing).
    npass = max(1, -(-maxc // MAXTOK))
    percap = -(-maxc // npass)
    ccap = max(32, -(-percap // 32) * 32)

    x_dq = _rtne4(hs).astype(np.float16)   # 4-sig-bit values: exact in fp16
    rw_flat = rw.reshape(-1)

    def stage_w(mat_t, bias):
        # mat_t: [K <= 2880, N <= 2880] contraction-major -> (main, ntail, ktail)
        w = np.zeros((2881, 2880), np.float16)
        w[: mat_t.shape[0], : mat_t.shape[1]] = mat_t.astype(np.float16)
        w[2880, : bias.shape[0]] = bias.astype(np.float16)
        main = w[: KF * P].reshape(KF, P, 2880)          # [kt, kp, n]
        wm = np.ascontiguousarray(
            main[:, :, : KF * P].reshape(KF, P, KF, P).transpose(2, 1, 0, 3)
        )                                                 # [nt, kp, kt, np]
        wn = np.ascontiguousarray(main[:, :, KF * P :].transpose(1, 0, 2))
        wk = np.ascontiguousarray(w[KF * P :])            # [65, 2880]
        return wm, wn, wk

    passes, assigns = [], []
    weights = []
    for e in range(NE):
        gm, gn, gk = stage_w(gup[e, 0::2, :].T, gub[e, 0::2])
        um, un, uk = stage_w(gup[e, 1::2, :].T, gub[e, 1::2])
        dm, dnn, dk = stage_w(dn[e].T, dnb[e])
        weights.append(dict(
            w1m=np.ascontiguousarray(np.stack([gm, um], axis=2)),
            w1n=np.ascontiguousarray(np.stack([gn, un], axis=1)),
            w1k=np.ascontiguousarray(np.stack([gk, uk], axis=1)),
            w2m=dm, w2n=dnn, w2k=dk,
        ))
    for p in range(npass):
        in_maps, passigns = [], []
        for e in range(NE):
            a_all = order[starts[e] : starts[e + 1]]
            a = a_all[p * ccap : (p + 1) * ccap]
            toks = a // topk
            ce = len(a)
            passigns.append((a, toks))

            xt_full = x_dq[toks].T                        # [2880, ce]
            xt = np.zeros((P, KF, ccap), np.float16)
            xt[:, :, :ce] = xt_full[: KF * P].reshape(KF, P, ce).transpose(1, 0, 2)
            xtl = np.zeros((KTAIL, ccap), np.float16)
            xtl[:NWT, :ce] = xt_full[KF * P :]
            xtl[NWT, :] = np.float16(1.0)

            wr_rep = np.zeros((P, ccap), np.float32)
            wr_rep[:, :ce] = rw_flat[a][None, :]

            in_maps.append(dict(xt=xt, xtl=xtl, wr=wr_rep, **weights[e]))
        passes.append(in_maps)
        assigns.append(passigns)

    nc = _build(ccap)
    return nc, passes, assigns, T


def kernel(**inputs):
    nc, passes, assigns, T = _stage(inputs)
    out = np.zeros((T, H), np.float32)
    for in_maps, passigns in zip(passes, assigns):
        res = run_bass_kernel_spmd(nc, in_maps, list(range(NE)))
        for e in range(NE):
            a, toks = passigns[e]
            if len(a):
                yt = res.results[e]["y"]            # [P, NT, ccap]
                yt = yt.transpose(1, 0, 2).reshape(NT * P, -1)[:H, : len(a)]
                np.add.at(out, toks, yt.T.astype(np.float32))
    return out


# revision 30
# speedup vs baseline: 1.0609x; 1.0070x over previous
"""GPT-OSS MoE experts kernel for Trainium2 (8 NeuronCores, expert-parallel).

Strategy
--------
- Expert-parallel: core e owns expert e's weights (1/8 of total weight bytes,
  read exactly once -> memory-bound). Host does routing (gather tokens per
  expert), weight re-staging (slice expert, transpose to contraction-major
  [K, N] tile layout, cast fp16), and the final scatter-add combine. No
  collectives needed.
- The reference's per-32-block fp8 quant-dequant collapses exactly to
  "round each element to 4 significant bits (RTNE)": the block scale is a
  power of two (mantissa rounding is scale-invariant) and the +-448 clip can
  never bind by construction. On device this is 3 VectorE ops (Veltkamp
  split); the 4-significant-bit activation values are then EXACT in fp16.
- fp16 weights round at 2^-11; end-to-end error vs the f32 reference is
  ~7e-3 absmax-rel - dominated by quantization-boundary flips from layer-1
  perturbations - and fp16 halves the weight traffic of this DMA-bound
  kernel.
- Form-B matmuls: weight [K-part, n] tiles are the STATIONARY operand, ALL
  tokens ride the moving free dim (ccap <= 512). Outputs land output-major
  ([n, tokens]), which feeds layer 2 directly - no on-chip transposes.
- No padding waste: the contraction splits into 22 full 128-tiles plus a
  single [65, 2880] "k-tail strip" per matrix (rows 2816..2879 + the bias
  row, with the activations carrying a ones row at tail partition 64); the
  output dim splits into 22 full 128-tiles plus one 64-wide tail tile.
  Weight bytes per core = 3 * 2881*2880*2B = 49.8 MB (vs 52.0 padded).
- gate and up slabs are staged interleaved so each layer-1 n-tile is ONE
  contiguous ~1.4 MB DMA (11264 B per partition line).
- y accumulates in one resident SBUF tile, stored in three big chunks on the
  Scalar HWDGE ring: store completions must never gate the (Sync-ring)
  weight-load stream, because the 8 shared DMA-completion lanes round-robin
  over ALL HWDGE DMAs and a lingering store blocks later loads on its lane.
- Measured: the weight stream sustains ~420 GB/s through layer 1 and the
  whole-kernel DMA averages ~358 GB/s (the per-NC HBM share); ~6 us fixed
  program preamble and ~9 us drain/barrier postamble bracket the stream.
"""

import functools
import sys

sys.path.insert(0, "/opt/trn_rl_repo")

import numpy as np

import concourse.bass as bass  # noqa: F401
import concourse.mybir as mybir
import concourse.tile as tile
from concourse import bacc
from concourse.bass_utils import run_bass_kernel_spmd

P = 128
H = 2880          # hidden dim
II = 2880         # intermediate dim (gate/up width)
NE = 8            # experts == cores
KF = 22           # full 128-tiles over the contraction dim (22*128 = 2816)
KTAIL = 65        # k-tail strip: rows 2816..2879 + bias/ones row
NT = 23           # output tiles: 22 full + one 64-wide tail
NWT = 64          # width of the output tail tile
VC = float(2 ** 20 + 1)   # Veltkamp constant: RTNE to 4 significant bits
MAXTOK = 512              # moving free-dim (= PSUM f32 bank) limit

f32 = mybir.dt.float32
f16 = mybir.dt.float16
AF = mybir.ActivationFunctionType
ALU = mybir.AluOpType


def _rtne4(x):
    """Round f32 elements to 4 significant bits, RTNE (== reference
    quant_dequant_fp8 up to e4m3-subnormal leftovers)."""
    c = np.float32(VC)
    t = (x * c).astype(np.float32)
    return (t - (t - x)).astype(np.float32)


@functools.lru_cache(maxsize=4)
def _build(ccap):
    """Per-core Bass program; ccap = padded token capacity (<= MAXTOK)."""
    nc = bacc.Bacc(None, target_bir_lowering=False)

    xt_d = nc.declare_dram_parameter("xt", [P, KF, ccap], f16, isOutput=False)
    xtl_d = nc.declare_dram_parameter("xtl", [KTAIL, ccap], f16, isOutput=False)
    w1m_d = nc.declare_dram_parameter("w1m", [KF, P, 2, KF, P], f16, isOutput=False)
    w1n_d = nc.declare_dram_parameter("w1n", [P, 2, KF, NWT], f16, isOutput=False)
    w1k_d = nc.declare_dram_parameter("w1k", [KTAIL, 2, 2880], f16, isOutput=False)
    w2m_d = nc.declare_dram_parameter("w2m", [KF, P, KF, P], f16, isOutput=False)
    w2n_d = nc.declare_dram_parameter("w2n", [P, KF, NWT], f16, isOutput=False)
    w2k_d = nc.declare_dram_parameter("w2k", [KTAIL, 2880], f16, isOutput=False)
    wr_d = nc.declare_dram_parameter("wr", [P, ccap], f32, isOutput=False)
    y_d = nc.declare_dram_parameter("y", [P, NT, ccap], f16, isOutput=True)

    with tile.TileContext(nc) as tc:
        with (
            tc.tile_pool(name="consts", bufs=1) as consts,
            tc.tile_pool(name="w1slab", bufs=10) as w1pool,
            tc.tile_pool(name="w2slab", bufs=8) as w2pool,
            tc.tile_pool(name="tmp", bufs=2) as tmp,
            tc.tile_pool(name="psum", bufs=4, space="PSUM") as psum,
        ):
            # resident tensors; the input prologue rides the Scalar HWDGE
            # ring so the Sync ring starts streaming weight slabs at t=0
            xts = consts.tile([P, KF, ccap], f16, tag="xt", name="xt")
            nc.scalar.dma_start(xts, xt_d[:])
            xtl = consts.tile([KTAIL, ccap], f16, tag="xtl", name="xtl")
            nc.scalar.dma_start(xtl, xtl_d[:])
            wrep = consts.tile([P, ccap], f32, tag="wrep", name="wrep")
            nc.scalar.dma_start(wrep, wr_d[:])
            kt1 = consts.tile([KTAIL, 2, 2880], f16, tag="kt1", name="kt1")
            interT = consts.tile([P, KF, ccap], f16, tag="interT", name="interT")
            intertail = consts.tile([KTAIL, ccap], f16, tag="intertail",
                                    name="intertail")
            ysb = consts.tile([P, NT, ccap], f16, tag="ysb", name="ysb")

            # HAM warmup while the first slabs + xt stream in
            wtile = consts.tile([P, P], f16, tag="wtile", name="wtile")
            nc.vector.memset(wtile, 0.25)
            wup = psum.tile([P, ccap], f32, tag="ps_g", name="wup")
            for _ in range(48):
                nc.tensor.matmul(wup[:, :P], wtile, wtile,
                                 start=True, stop=True, skip_group_check=True)

            # ---- layer 1 + swiglu + rtne4, one n-tile at a time ----
            for nt in range(NT):
                nw = P if nt < KF else NWT
                if nt < KF:
                    slab = w1pool.tile([P, 2, KF, P], f16, tag="w1slab",
                                       name="w1slab")
                    nc.sync.dma_start(slab, w1m_d[nt])
                else:
                    slab = w1pool.tile([P, 2, KF, NWT], f16, tag="w1slab",
                                       name="w1slab")
                    nc.sync.dma_start(slab, w1n_d[:])
                if nt == 0:
                    # FIFO position: after slab0, before slab1 (needed by the
                    # 45th MM of iteration 0 -> arrives in time either way)
                    nc.sync.dma_start(kt1, w1k_d[:])
                gps = psum.tile([nw, ccap], f32, tag="ps_g", name="ps_g")
                ups = psum.tile([nw, ccap], f32, tag="ps_u", name="ps_u")
                for k in range(KF):
                    nc.tensor.matmul(gps, slab[:, 0, k, :], xts[:, k, :],
                                     start=(k == 0), stop=False)
                    nc.tensor.matmul(ups, slab[:, 1, k, :], xts[:, k, :],
                                     start=(k == 0), stop=False)
                nc.tensor.matmul(gps, kt1[:, 0, nt * P : nt * P + nw], xtl,
                                 start=False, stop=True)
                nc.tensor.matmul(ups, kt1[:, 1, nt * P : nt * P + nw], xtl,
                                 start=False, stop=True)
                # swiglu: gate=min(G,7); up1=clip(U,-7,7)+1; x=gate*sig(1.702g)*up1
                gate = tmp.tile([nw, ccap], f32, tag="t_gate", name="t_gate")
                nc.vector.tensor_scalar_min(gate, gps, 7.0)
                sig = tmp.tile([nw, ccap], f32, tag="t_sig", name="t_sig")
                nc.scalar.activation(sig, gate, AF.Sigmoid, scale=1.702)
                up1 = tmp.tile([nw, ccap], f32, tag="t_up", name="t_up")
                nc.vector.tensor_scalar(up1, ups, 1.0, -6.0, ALU.add, ALU.max)
                nc.vector.tensor_scalar_min(up1, up1, 8.0)
                nc.vector.tensor_mul(gate, gate, sig)          # gate*sig
                xv = tmp.tile([nw, ccap], f32, tag="t_xv", name="t_xv")
                nc.vector.tensor_mul(xv, gate, up1)            # x = swiglu
                tv = tmp.tile([nw, ccap], f32, tag="t_tv", name="t_tv")
                nc.vector.tensor_scalar_mul(tv, xv, VC)        # t = x*c
                nc.vector.tensor_sub(xv, tv, xv)               # d = t-x
                dst = interT[:, nt, :] if nt < KF else intertail[:NWT, :]
                nc.vector.tensor_sub(dst, tv, xv)              # rtne4 = t-d
            # layer-2 ones row rides tail partition 64 (pairs w2's bias row)
            nc.vector.memset(intertail[NWT : NWT + 1, :], 1.0)

            # kt2 load sits here in FIFO order: after all w1 slabs, before w2
            kt2 = consts.tile([KTAIL, 2880], f16, tag="kt2", name="kt2")
            nc.sync.dma_start(kt2, w2k_d[:])

            # ---- layer 2 + routing-weight scale ----
            YSPLIT = (11, 19, 22)   # store y in 3 chunks; last one is tiny
            for ht in range(NT):
                nw = P if ht < KF else NWT
                if ht < KF:
                    slab2 = w2pool.tile([P, KF, P], f16, tag="w2slab",
                                        name="w2slab")
                    nc.sync.dma_start(slab2, w2m_d[ht])
                else:
                    slab2 = w2pool.tile([P, KF, NWT], f16, tag="w2slab",
                                        name="w2slab")
                    nc.sync.dma_start(slab2, w2n_d[:])
                yps = psum.tile([nw, ccap], f32, tag="ps_g", name="ps_g")
                for k in range(KF):
                    nc.tensor.matmul(yps, slab2[:, k, :], interT[:, k, :],
                                     start=(k == 0), stop=False)
                nc.tensor.matmul(yps, kt2[:, ht * P : ht * P + nw], intertail,
                                 start=False, stop=True)
                nc.vector.tensor_mul(ysb[:nw, ht, :], yps, wrep[:nw, :])
                if ht in YSPLIT:
                    lo = 0 if ht <= YSPLIT[0] else YSPLIT[YSPLIT.index(ht) - 1] + 1
                    nc.scalar.dma_start(y_d[:, lo : ht + 1, :],
                                        ysb[:, lo : ht + 1, :])

    nc.finalize()
    return nc


def _stage(inputs):
    """Host-side routing + weight re-staging. Returns (nc, passes, assigns, T)."""
    hs = np.ascontiguousarray(np.asarray(inputs["hidden_states"], dtype=np.float32))
    ri = np.asarray(inputs["router_indices"]).astype(np.int64)
    rw = np.asarray(inputs["routing_weights"], dtype=np.float32)
    gup = np.asarray(inputs["gate_up_proj"], dtype=np.float32)
    gub = np.asarray(inputs["gate_up_proj_bias"], dtype=np.float32)
    dn = np.asarray(inputs["down_proj"], dtype=np.float32)
    dnb = np.asarray(inputs["down_proj_bias"], dtype=np.float32)

    T = hs.shape[0]
    topk = ri.shape[1]

    flat_e = ri.reshape(-1)
    order = np.argsort(flat_e, kind="stable")
    counts = np.bincount(flat_e, minlength=NE)
    starts = np.zeros(NE + 1, np.int64)
    starts[1:] = np.cumsum(counts)
    maxc = int(counts.max())
    # Each pass handles up to MAXTOK tokens per expert (seed-0 loads are ~142,
    # so this is a single pass; multiple passes only for pathological routing).
    npass = max(1, -(-maxc // MAXTOK))
    percap = -(-maxc // npass)
    ccap = max(32, -(-percap // 32) * 32)

    x_dq = _rtne4(hs).astype(np.float16)   # 4-sig-bit values: exact in fp16
    rw_flat = rw.reshape(-1)

    def stage_w(mat_t, bias):
        # mat_t: [K <= 2880, N <= 2880] contraction-major -> (main, ntail, ktail)
        w = np.zeros((2881, 2880), np.float16)
        w[: mat_t.shape[0], : mat_t.shape[1]] = mat_t.astype(np.float16)
        w[2880, : bias.shape[0]] = bias.astype(np.float16)
        main = w[: KF * P].reshape(KF, P, 2880)          # [kt, kp, n]
        wm = np.ascontiguousarray(
            main[:, :, : KF * P].reshape(KF, P, KF, P).transpose(2, 1, 0, 3)
        )                                                 # [nt, kp, kt, np]
        wn = np.ascontiguousarray(main[:, :, KF * P :].transpose(1, 0, 2))
        wk = np.ascontiguousarray(w[KF * P :])            # [65, 2880]
        return wm, wn, wk

    passes, assigns = [], []
    weights = []
    for e in range(NE):
        gm, gn, gk = stage_w(gup[e, 0::2, :].T, gub[e, 0::2])
        um, un, uk = stage_w(gup[e, 1::2, :].T, gub[e, 1::2])
        dm, dnn, dk = stage_w(dn[e].T, dnb[e])
        weights.append(dict(
            w1m=np.ascontiguousarray(np.stack([gm, um], axis=2)),
            w1n=np.ascontiguousarray(np.stack([gn, un], axis=1)),
            w1k=np.ascontiguousarray(np.stack([gk, uk], axis=1)),
            w2m=dm, w2n=dnn, w2k=dk,
        ))
    for p in range(npass):
        in_maps, passigns = [], []
        for e in range(NE):
            a_all = order[starts[e] : starts[e + 1]]
            a = a_all[p * ccap : (p + 1) * ccap]
            toks = a // topk
            ce = len(a)
            passigns.append((a, toks))

            xt_full = x_dq[toks].T                        # [2880, ce]
            xt = np.zeros((P, KF, ccap), np.float16)
            xt[:, :, :ce] = xt_full[: KF * P].reshape(KF, P, ce).transpose(1, 0, 2)
            xtl = np.zeros((KTAIL, ccap), np.float16)
            xtl[:NWT, :ce] = xt_full[KF * P :]
            xtl[NWT, :] = np.float16(1.0)

            wr_rep = np.zeros((P, ccap), np.float32)
            wr_rep[:, :ce] = rw_flat[a][None, :]

            in_maps.append(dict(xt=xt, xtl=xtl, wr=wr_rep, **weights[e]))
        passes.append(in_maps)
        assigns.append(passigns)

    nc = _build(ccap)
    return nc, passes, assigns, T


def kernel(**inputs):
    nc, passes, assigns, T = _stage(inputs)
    out = np.zeros((T, H), np.float32)
    for in_maps, passigns in zip(passes, assigns):
        res = run_bass_kernel_spmd(nc, in_maps, list(range(NE)))
        for e in range(NE):
            a, toks = passigns[e]
            if len(a):
                yt = res.results[e]["y"]            # [P, NT, ccap]
                yt = yt.transpose(1, 0, 2).reshape(NT * P, -1)[:H, : len(a)]
                np.add.at(out, toks, yt.T.astype(np.float32))
    return out
